# revision 27
# baseline (speedup 1.0000x reference)
"""Self-contained Trainium2 Bass kernel for nn_MultiHeadAttention_46969762349562.

Full fp32 inputs -> full fp32 output, distributed over 8 NeuronCores in two
SPMD stages (all FLOPs on device; host only slices/casts/concats/transposes):

  stage 1 (core = batch x 4-head group): Q/K/V projections (column-parallel
    weights, activations staged pre-transposed in bf16), attention with
    transposed score tiles [k_seq, q].  AV uses the transposed formulation
    out[q, vdim] = ex^T @ [V | ones]  (ex as the matmul stationary): 65-col
    AV matmuls at full PE utilization, half the PE cycles of the moving-ex
    form, with the softmax denominator free in column 64.  Scores are O(+-6)
    so no max-subtraction is needed.  exp runs on the ACT engine.  Because
    PSUM allows only ONE open accumulation region per bank, each pass's AV
    sweep (q4-outer over its two av banks) plus softmax normalization (on
    DVE; the denominator is a per-partition scalar in this layout) is
    deferred into the NEXT pass's timeline via gated tail generators; the
    exp tiles live in a 28-slot ring whose slot reuse is sequenced by tail
    progress labels.  Projections and per-head V are software-pipelined the
    same way with just-in-time requirements matched to the DMA arrival
    order; warmup matmuls keep the PE p-state hot through the DMA-bound
    startup.
  host reshard: head-sharded row-major x -> row-sharded x^T (slices+transpose).
  stage 2 (core = batch x 512-row chunk): output projection + bias only,
    group-outer accumulation so output DMA overlaps compute.
"""
import sys
for p in ('/opt/trn_rl_repo', '/root/.axon_site/_ro/trn_rl_repo'):
    if p not in sys.path:
        sys.path.append(p)
import numpy as np
import ml_dtypes
import concourse.bass as bass
import concourse.bacc as bacc
import concourse.mybir as mybir
from concourse import tile
from concourse.bass_utils import run_bass_kernel_spmd
from contextlib import ExitStack
from itertools import chain

dt = mybir.dt
AF = mybir.ActivationFunctionType
ALU = mybir.AluOpType
BF16 = ml_dtypes.bfloat16

B = 2
S = 2048
D = 1024
H = 16
DH = 64
HL = 4            # heads per core (stage 1)
DG = HL * DH      # 256 dims per head-group
NI = D // 128     # 8 contraction tiles
NKS = S // 128    # 16 key tiles
QP = 1024         # q-pass width
NQS = QP // 128   # 8 q sub-blocks per pass
CHUNK = S // 4    # 512 rows per stage-2 core
N_CORES = 8
NEX = 28          # exp-tile ring size (2 banks hold one open
                  # accumulation region each, so AV defers a full pass)

# Schraudolph exp on DVE: bf16bits(exp(x/8)) ~= int16(qk * A_DVE + B_DVE)
A_DVE = 128.0 / np.log(2.0) / 8.0
B_DVE = 16248.6               # 127*128 recentred for the sawtooth bias
ALT_PASSES = ()   # passes alternating exp between ACT and DVE (off: no gain)


def build_stage1(n_cores=8):
    nc = bacc.Bacc("TRN2", target_bir_lowering=False, debug=False, num_devices=n_cores)
    # [p, i, s] layouts: element (p, i, s) = x^T[128*i + p, s]
    xq = nc.declare_dram_parameter("xq", [128, NI, S], dt.bfloat16, isOutput=False)
    xk = nc.declare_dram_parameter("xk", [128, NI, S], dt.bfloat16, isOutput=False)
    xv = nc.declare_dram_parameter("xv", [128, NI, S], dt.bfloat16, isOutput=False)
    # wqkv[p, i, :] = (Wq.T | Wk.T | Wv.T)[128*i + p, :] (256 cols each)
    wqkv = nc.declare_dram_parameter("wqkv", [128, NI, 3 * DG], dt.bfloat16, isOutput=False)
    bqkT = nc.declare_dram_parameter("bqkT", [128, 4], dt.float32, isOutput=False)
    # bvb[p, r, :] = bv (same for all p, r): 4x-replicated for batched evictions
    bvb = nc.declare_dram_parameter("bvb", [128, 4, DG], dt.float32, isOutput=False)
    # normalized attention output: [qp*8+qs, p, c] = x[1024qp+128qs+p, c]
    xo = nc.declare_dram_parameter("xo", [16, 128, DG], dt.bfloat16, isOutput=True)

    with tile.TileContext(nc) as tc, ExitStack() as ctx:
        const = ctx.enter_context(tc.tile_pool(name="const", bufs=1))
        biasT = const.tile([128, 4], dt.float32, name="biasT", tag="biasT")
        bvb_sb = const.tile([128, 4, DG], dt.float32, name="bvb", tag="bvb")
        dum = const.tile([128, 512], dt.bfloat16, name="dum", tag="dum")
        nc.vector.memset(dum[:], 0.0)
        nc.sync.dma_start(biasT[:], bqkT[:])
        nc.sync.dma_start(bvb_sb[:], bvb[:])

        xpool = ctx.enter_context(tc.tile_pool(name="x", bufs=1))
        wq_sb = xpool.tile([128, NI, 3 * DG], dt.bfloat16, name="wq_sb", tag="w")
        xk_sb = xpool.tile([128, NI, S], dt.bfloat16, name="xk_sb", tag="xk")
        xq_sb = xpool.tile([128, NI, S], dt.bfloat16, name="xq_sb", tag="xq")
        xv_sb = xpool.tile([128, NI, S], dt.bfloat16, name="xv_sb", tag="xv")

        # staged input DMA: K weights+first keys, Q first window, then K rest
        # just-in-time for the exp pacing, V, and finally the second Q window
        nc.sync.dma_start(wq_sb[:, :, DG:2 * DG], wqkv[:, :, DG:2 * DG])
        nc.sync.dma_start(wq_sb[:, :, 0:DG], wqkv[:, :, 0:DG])
        nc.sync.dma_start(xk_sb[:, :, 0:512], xk[:, :, 0:512])
        nc.sync.dma_start(xq_sb[:, :, 0:512], xq[:, :, 0:512])
        nc.sync.dma_start(xq_sb[:, :, 512:1024], xq[:, :, 512:1024])
        nc.sync.dma_start(xk_sb[:, :, 512:1024], xk[:, :, 512:1024])
        nc.sync.dma_start(xk_sb[:, :, 1024:1536], xk[:, :, 1024:1536])
        nc.sync.dma_start(xk_sb[:, :, 1536:2048], xk[:, :, 1536:2048])
        nc.sync.dma_start(wq_sb[:, :, 2 * DG:3 * DG], wqkv[:, :, 2 * DG:3 * DG])
        nc.sync.dma_start(xv_sb[:, :, 0:QP], xv[:, :, 0:QP])
        nc.sync.dma_start(xv_sb[:, :, QP:S], xv[:, :, QP:S])
        nc.sync.dma_start(xq_sb[:, :, QP:S], xq[:, :, QP:S])

        kT = [xpool.tile([128, S], dt.bfloat16, name=f"kT{p}", tag=f"kT{p}") for p in range(2)]
        qT = [xpool.tile([128, S], dt.bfloat16, name=f"qT{p}", tag=f"qT{p}") for p in range(2)]
        vext = xpool.tile([128, NKS, HL, DH + 1], dt.bfloat16, name="vext", tag="vext")
        nc.vector.memset(vext[:, :, :, DH:DH + 1], 1.0)
        xo_big = [xpool.tile([128, NQS, DG], dt.bfloat16, name=f"xo{qp}", tag=f"xo{qp}")
                  for qp in range(2)]

        pp = ctx.enter_context(tc.tile_pool(name="pp", bufs=2, space="PSUM"))
        avp = ctx.enter_context(tc.tile_pool(name="avp", bufs=1, space="PSUM"))
        pj = ctx.enter_context(tc.tile_pool(name="pj", bufs=2, space="PSUM"))
        exps = ctx.enter_context(tc.tile_pool(name="exps", bufs=NEX))
        ex16s = ctx.enter_context(tc.tile_pool(name="ex16s", bufs=3))
        rcps = ctx.enter_context(tc.tile_pool(name="rcps", bufs=2))

        def gen_warmup(n, lhs_ap):
            # keeps the PE p-state hot; lhs_ap gates the block on a DMA
            for _ in range(n):
                ps = pj.tile([128, 512], dt.float32, name="pjt", tag="pjt")
                nc.tensor.matmul(ps[:], lhsT=lhs_ap, rhs=dum[:],
                                 start=True, stop=True)
                yield None

        def gen_proj_chunk(xt, dst, wcol, bias_col, label, c):
            # dst[:, 512c:512c+512] = (W block).T @ x chunk + bias
            ps = pj.tile([128, 512], dt.float32, name="pjt", tag="pjt")
            for i in range(NI):
                nc.tensor.matmul(
                    ps[:], lhsT=wq_sb[:, i, wcol:wcol + 128],
                    rhs=xt[:, i, 512 * c:512 * (c + 1)],
                    start=(i == 0), stop=(i == NI - 1))
                yield None
            nc.vector.tensor_scalar_add(
                dst[:, 512 * c:512 * (c + 1)], ps[:],
                biasT[:, bias_col:bias_col + 1])
            yield (label, c + 1)

        # per-pass state shared with the deferred tail generators
        tails = {}

        def normalize(av, hl, qp, mix=False):
            for lohi in range(2):
                rcp = rcps.tile([128, 4], dt.float32, name="rcp", tag="rcp")
                nc.vector.reciprocal(rcp[:], av[lohi][:, :, DH:DH + 1])
                for q4 in range(4):
                    qs = 4 * lohi + q4
                    dst = xo_big[qp][:, qs, DH * hl:DH * (hl + 1)]
                    src = av[lohi][:, q4, 0:DH]
                    sc = rcp[:, q4:q4 + 1]
                    if mix and q4 % 2 == 1:
                        nc.scalar.mul(dst, src, sc)
                    else:
                        nc.vector.tensor_scalar_mul(dst, src, sc)
                if hl == HL - 1:
                    # dst AP reordered to (p, qs, c) so the SBUF source AP
                    # keeps its partition dim first (walrus requirement)
                    nc.sync.dma_start(
                        xo[8 * qp + 4 * lohi:8 * qp + 4 * lohi + 4, :, :]
                        .transpose([1, 0, 2]),
                        xo_big[qp][:, 4 * lohi:4 * lohi + 4, :])

        def v_unit4(hl, kb):
            # vext[:, 4kb:4kb+4, hl, 0:64] = head hl V for 4 key tiles.
            # j-outer: one open psum accumulation region per bank at a time.
            ps = pj.tile([128, 512], dt.float32, name="pjt", tag="pjt")
            for j in range(4):
                ks = 4 * kb + j
                for i in range(NI):
                    nc.tensor.matmul(
                        ps[:, DH * j:DH * (j + 1)],
                        lhsT=xv_sb[:, i, 128 * ks:128 * (ks + 1)],
                        rhs=wq_sb[:, i, 2 * DG + DH * hl:2 * DG + DH * (hl + 1)],
                        start=(i == 0), stop=(i == NI - 1))
            nc.vector.tensor_tensor(
                vext[:, 4 * kb:4 * kb + 4, hl, 0:DH], ps[:, 0:4 * DH],
                bvb_sb[:, :, DH * hl:DH * (hl + 1)], ALU.add)

        def gen_vh(hl):
            for kb in range(4):
                v_unit4(hl, kb)
                yield (f"vh{hl}", 4 * (kb + 1))

        def gen_tailk(k):
            # pass k's AV sweep + softmax normalize, absorbed into pass k+1.
            # q4-outer so each psum bank has ONE open accumulation region at a
            # time (hardware requirement); the two av banks run in parallel.
            # Progress label t{k}: count 16*q4 + ks + 1; ex[ks] is free once
            # count >= 48 + ks + 1 (its q4=3 read).
            if k == 0:
                for kb in range(4):
                    v_unit4(0, kb)
                    yield ("vh0", 4 * (kb + 1))
            av, exs, hl, qp = tails[k]
            for q4 in range(4):
                for ks in range(NKS):
                    nc.tensor.matmul(
                        av[0][:, q4, :],
                        lhsT=exs[ks][:, 128 * q4:128 * (q4 + 1)],
                        rhs=vext[:, ks, hl, :],
                        start=(ks == 0), stop=(ks == NKS - 1))
                    nc.tensor.matmul(
                        av[1][:, q4, :],
                        lhsT=exs[ks][:, 128 * (q4 + 4):128 * (q4 + 5)],
                        rhs=vext[:, ks, hl, :],
                        start=(ks == 0), stop=(ks == NKS - 1))
                    yield (f"t{k}", 16 * q4 + ks + 1)
            normalize(av, hl, qp, mix=(k == 7))
            yield (f"t{k}", 65)

        class Filler:
            def __init__(self, units):
                self.units = units
                self.done = {}
                self.empty = False
                self.gate = None       # blocked on this closed gate
                self.open = set()

            def pull(self, n=1):
                for _ in range(n):
                    if self.gate is not None:
                        if self.gate not in self.open:
                            return
                        self.gate = None
                    try:
                        lab = next(self.units)
                    except StopIteration:
                        self.empty = True
                        return
                    if lab is None:
                        continue
                    if lab[0] == "GATE":
                        if lab[1] not in self.open:
                            self.gate = lab[1]
                            return
                    else:
                        self.done[lab[0]] = lab[1]

            def open_gate(self, k):
                self.open.add(k)

            def require(self, key, count):
                while self.done.get(key, 0) < count:
                    assert not self.empty, f"filler exhausted before {key}={count}"
                    assert not (self.gate is not None and self.gate not in self.open), \
                        f"require {key}={count} blocked by gate {self.gate}"
                    self.pull(1)

        def gen_gate(k):
            yield ("GATE", k)

        def k_chunk(p, c):
            return gen_proj_chunk(xk_sb, kT[p], DG + 128 * p, 2 + p, f"k{p}", c)

        def q_chunk(p, c):
            return gen_proj_chunk(xq_sb, qT[p], 128 * p, p, f"q{p}", c)

        fill = Filler(chain(
            gen_warmup(10, dum[:, 0:128]),
            gen_warmup(15, wq_sb[:, 0, DG:DG + 128]),
            k_chunk(0, 0), q_chunk(0, 0), q_chunk(0, 1),
            k_chunk(0, 1), k_chunk(0, 2), k_chunk(0, 3),
            k_chunk(1, 0), k_chunk(1, 1), k_chunk(1, 2), k_chunk(1, 3),
            gen_gate(0), gen_tailk(0),
            gen_vh(1), q_chunk(1, 0), q_chunk(1, 1),
            gen_gate(1), gen_tailk(1),
            gen_vh(2), q_chunk(0, 2), q_chunk(0, 3),
            gen_gate(2), gen_tailk(2),
            gen_vh(3), q_chunk(1, 2), q_chunk(1, 3),
            gen_gate(3), gen_tailk(3),
            gen_gate(4), gen_tailk(4),
            gen_gate(5), gen_tailk(5),
            gen_gate(6), gen_tailk(6),
            gen_gate(7), gen_tailk(7),
        ))

        def emit_exp(qk, dve):
            if dve:
                exi = ex16s.tile([128, QP], dt.int16, name="exi", tag="exi")
                nc.vector.tensor_scalar(
                    exi[:], qk[:], A_DVE, B_DVE, ALU.mult, ALU.add)
                return exi.bitcast(dt.bfloat16)
            ext = exps.tile([128, QP], dt.bfloat16, name="ex", tag="ex")
            nc.scalar.activation(ext[:], qk[:], AF.Exp, scale=0.125)
            return ext

        def emit_qk(p, lo, qp, ks):
            qk = pp.tile([128, QP], dt.float32, name="qkt", tag="qkt")
            for nh in range(2):
                nc.tensor.matmul(
                    qk[:, 512 * nh:512 * (nh + 1)],
                    lhsT=kT[p][lo:lo + 64, 128 * ks:128 * (ks + 1)],
                    rhs=qT[p][lo:lo + 64, QP * qp + 512 * nh:QP * qp + 512 * nh + 512],
                    start=True, stop=True)
            return qk

        def attention_pass(idx, hl, qp):
            p, half = divmod(hl, 2)
            lo = 64 * half
            fill.require(f"q{p}", 2 * (qp + 1))
            av_lo = avp.tile([128, 4, DH + 1], dt.float32, name="av_lo", tag="av_lo")
            av_hi = avp.tile([128, 4, DH + 1], dt.float32, name="av_hi", tag="av_hi")
            exs = []
            tails[idx] = ([av_lo, av_hi], exs, hl, qp)
            for ks in range(NKS):
                fill.require(f"k{p}", min(ks // 4 + 1, 4))
                n = 16 * idx + ks
                if n - NEX >= 0:
                    ep, eks = divmod(n - NEX, NKS)
                    fill.require(f"t{ep}", 48 + eks + 1)
                qk = emit_qk(p, lo, qp, ks)
                exs.append(emit_exp(qk, idx in ALT_PASSES and ks % 2 == 1))
                fill.pull(5 if idx == 0 and ks >= 6 else 4)
            fill.open_gate(idx)

        order = [(0, 0), (1, 0), (2, 0), (3, 0), (0, 1), (1, 1), (2, 1), (3, 1)]
        for idx, (hl, qp) in enumerate(order):
            attention_pass(idx, hl, qp)
        while not fill.empty:
            fill.pull(4)

    nc.compile()
    return nc


def build_stage2(n_cores=8):
    nc = bacc.Bacc("TRN2", target_bir_lowering=False, debug=False, num_devices=n_cores)
    # xT[p, i, r] = x^T[128i + p, r] for this core's CHUNK rows
    xT = nc.declare_dram_parameter("xT", [128, NI, CHUNK], dt.bfloat16, isOutput=False)
    # woT[p, i, c] = Wo.T[128i + p, c]
    woT = nc.declare_dram_parameter("woT", [128, NI, D], dt.bfloat16, isOutput=False)
    bo = nc.declare_dram_parameter("bo", [1, D], dt.bfloat16, isOutput=False)
    out = nc.declare_dram_parameter("out", [CHUNK, D], dt.float32, isOutput=True)

    with tile.TileContext(nc) as tc, ExitStack() as ctx:
        pool = ctx.enter_context(tc.tile_pool(name="p2", bufs=1))
        ones_k1 = pool.tile([1, 512], dt.bfloat16, name="ones_k1", tag="ones")
        nc.vector.memset(ones_k1[:], 1.0)
        bo_sb = pool.tile([1, D], dt.bfloat16, name="bo_sb", tag="bo")
        nc.sync.dma_start(bo_sb[:], bo[:])
        xT_sb = pool.tile([128, NI, CHUNK], dt.bfloat16, name="xT_sb", tag="xT")
        wo_sb = pool.tile([128, NI, D], dt.bfloat16, name="wo_sb", tag="wo")
        for c in range(4):
            nc.sync.dma_start(xT_sb[:, 2 * c:2 * c + 2, :], xT[:, 2 * c:2 * c + 2, :])
            nc.sync.dma_start(wo_sb[:, 2 * c:2 * c + 2, :], woT[:, 2 * c:2 * c + 2, :])

        psp = ctx.enter_context(tc.tile_pool(name="psp", bufs=3, space="PSUM"))
        # PE p-state warmup while the first DMA chunks land
        for _ in range(14):
            ps = psp.tile([128, 512], dt.float32, name="fc", tag="fc")
            nc.tensor.matmul(ps[:], lhsT=ones_k1[0:1, 0:128], rhs=ones_k1[:],
                             start=True, stop=True)

        def mm(ps, g, t):
            rt, nh = divmod(g, 2)
            nc.tensor.matmul(
                ps[:], lhsT=xT_sb[:, t, 128 * rt:128 * (rt + 1)],
                rhs=wo_sb[:, t, 512 * nh:512 * (nh + 1)],
                start=(t == 0), stop=False)

        def finish(ps, g):
            rt, nh = divmod(g, 2)
            nc.tensor.matmul(
                ps[:], lhsT=ones_k1[0:1, 0:128],
                rhs=bo_sb[0:1, 512 * nh:512 * (nh + 1)],
                start=False, stop=True)
            os = pool.tile([128, 512], dt.float32, name=f"os{g}", tag=f"os{g % 4}",
                           bufs=1)
            if g % 2 == 0:
                nc.scalar.copy(os[:], ps[:])
            else:
                nc.vector.tensor_copy(os[:], ps[:])
            nc.sync.dma_start(
                out[128 * rt:128 * (rt + 1), 512 * nh:512 * (nh + 1)], os[:])

        # groups 0,1 interleaved across t (overlaps the input DMA window),
        # then groups 2..7 group-outer so output DMA overlaps compute
        ps0 = psp.tile([128, 512], dt.float32, name="fc", tag="fc")
        ps1 = psp.tile([128, 512], dt.float32, name="fc", tag="fc")
        for t in range(NI):
            mm(ps0, 0, t)
            mm(ps1, 1, t)
        finish(ps0, 0)
        finish(ps1, 1)
        for g in range(2, 8):
            ps = psp.tile([128, 512], dt.float32, name="fc", tag="fc")
            for t in range(NI):
                mm(ps, g, t)
            finish(ps, g)

    nc.compile()
    return nc


def _fold(a):
    """[Dfull, N] -> [128, Dfull//128, N] with element (p, i, n) = a[128i+p, n]."""
    d, n = a.shape
    return np.ascontiguousarray(a.reshape(d // 128, 128, n).transpose(1, 0, 2))


def stage1_inputs(inputs):
    """core c = (b, g): b = c // 4, g = c % 4 (head-group of 4 heads)."""
    xt = {}
    for nm in ('query', 'key', 'value'):
        for b in range(B):
            xt[(nm, b)] = _fold(np.ascontiguousarray(
                np.asarray(inputs[nm])[b].T).astype(BF16))
    maps = []
    for c in range(8):
        b, g = divmod(c, 4)
        rows = slice(DG * g, DG * (g + 1))
        bq = np.asarray(inputs['bq'])[rows].astype(np.float32)
        bk = np.asarray(inputs['bk'])[rows].astype(np.float32)
        bv = np.asarray(inputs['bv'])[rows].astype(np.float32)
        bqkT = np.stack([bq[0:128], bq[128:256], bk[0:128], bk[128:256]], axis=1)
        bvb = np.broadcast_to(bv[None, None, :], (128, 4, DG)).copy()
        w = np.concatenate([
            np.asarray(inputs['Wq'])[rows].T,
            np.asarray(inputs['Wk'])[rows].T,
            np.asarray(inputs['Wv'])[rows].T], axis=1).astype(BF16)
        maps.append({
            'xq': xt[('query', b)], 'xk': xt[('key', b)], 'xv': xt[('value', b)],
            'wqkv': _fold(np.ascontiguousarray(w)),
            'bqkT': bqkT, 'bvb': bvb,
        })
    return maps


def stage2_inputs(stage1_results, inputs):
    """stage1_results: list of 8 dicts with 'xo' [16, 128, 256] bf16 (normalized)."""
    woT = _fold(np.ascontiguousarray(np.asarray(inputs['Wo']).T).astype(BF16))
    bo = np.asarray(inputs['bo'])[None, :].astype(BF16)
    xrows_all = {}
    for cc in range(8):
        xrows_all[cc] = np.asarray(stage1_results[cc]['xo']).reshape(S, DG)
    maps = []
    for c in range(8):
        b, j = divmod(c, 4)
        rows = slice(CHUNK * j, CHUNK * (j + 1))
        xrows = np.concatenate(
            [xrows_all[4 * b + g][rows] for g in range(4)], axis=1)  # [512, 1024]
        maps.append({
            'xT': _fold(np.ascontiguousarray(xrows.T)),
            'woT': woT, 'bo': bo,
        })
    return maps


def assemble_output(stage2_results):
    out = np.empty((B, S, D), dtype=np.float32)
    for c in range(8):
        b, j = divmod(c, 4)
        out[b, CHUNK * j:CHUNK * (j + 1), :] = np.asarray(stage2_results[c]['out'])
    return out


_CACHE = {}


def _programs():
    if 'nc1' not in _CACHE:
        _CACHE['nc1'] = build_stage1(N_CORES)
        _CACHE['nc2'] = build_stage2(N_CORES)
    return _CACHE['nc1'], _CACHE['nc2']


def kernel(**inputs):
    nc1, nc2 = _programs()
    core_ids = list(range(N_CORES))
    s1_maps = stage1_inputs(inputs)
    r1 = run_bass_kernel_spmd(nc1, s1_maps, core_ids).results
    s2_maps = stage2_inputs(r1, inputs)
    r2 = run_bass_kernel_spmd(nc2, s2_maps, core_ids).results
    return assemble_output(r2)


# revision 28
# speedup vs baseline: 1.0143x; 1.0143x over previous
"""Self-contained Trainium2 Bass kernel for nn_MultiHeadAttention_46969762349562.

Full fp32 inputs -> full fp32 output, distributed over 8 NeuronCores in two
SPMD stages (all FLOPs on device; host only slices/casts/concats/transposes):

  stage 1 (core = batch x 4-head group): Q/K/V projections (column-parallel
    weights, activations staged pre-transposed in bf16), attention with
    transposed score tiles [k_seq, q].  AV uses the transposed formulation
    out[q, vdim] = ex^T @ [V | ones]  (ex as the matmul stationary): 65-col
    AV matmuls at full PE utilization, half the PE cycles of the moving-ex
    form, with the softmax denominator free in column 64.  Scores are O(+-6)
    so no max-subtraction is needed.  exp runs on the ACT engine.  Because
    PSUM allows only ONE open accumulation region per bank, each pass's AV
    sweep (q4-outer over its two av banks) plus softmax normalization (on
    DVE; the denominator is a per-partition scalar in this layout) is
    deferred into the NEXT pass's timeline via gated tail generators; the
    exp tiles live in a 28-slot ring whose slot reuse is sequenced by tail
    progress labels.  Projections and per-head V are software-pipelined the
    same way with just-in-time requirements matched to the DMA arrival
    order; warmup matmuls keep the PE p-state hot through the DMA-bound
    startup.
  host reshard: head-sharded row-major x -> row-sharded x^T (slices+transpose).
  stage 2 (core = batch x 512-row chunk): output projection + bias only,
    group-outer accumulation so output DMA overlaps compute.
"""
import sys
for p in ('/opt/trn_rl_repo', '/root/.axon_site/_ro/trn_rl_repo'):
    if p not in sys.path:
        sys.path.append(p)
import numpy as np
import ml_dtypes
import concourse.bass as bass
import concourse.bacc as bacc
import concourse.mybir as mybir
from concourse import tile
from concourse.bass_utils import run_bass_kernel_spmd
from contextlib import ExitStack
from itertools import chain

dt = mybir.dt
AF = mybir.ActivationFunctionType
ALU = mybir.AluOpType
BF16 = ml_dtypes.bfloat16

B = 2
S = 2048
D = 1024
H = 16
DH = 64
HL = 4            # heads per core (stage 1)
DG = HL * DH      # 256 dims per head-group
NI = D // 128     # 8 contraction tiles
NKS = S // 128    # 16 key tiles
QP = 1024         # q-pass width
NQS = QP // 128   # 8 q sub-blocks per pass
CHUNK = S // 4    # 512 rows per stage-2 core
N_CORES = 8
NEX = 28          # exp-tile ring size (2 banks hold one open
                  # accumulation region each, so AV defers a full pass)

# Schraudolph exp on DVE: bf16bits(exp(x/8)) ~= int16(qk * A_DVE + B_DVE)
A_DVE = 128.0 / np.log(2.0) / 8.0
B_DVE = 16248.6               # 127*128 recentred for the sawtooth bias
ALT_PASSES = ()   # passes alternating exp between ACT and DVE (off: no gain)


def build_stage1(n_cores=8):
    nc = bacc.Bacc("TRN2", target_bir_lowering=False, debug=False, num_devices=n_cores)
    # [p, i, s] layouts: element (p, i, s) = x^T[128*i + p, s]
    xq = nc.declare_dram_parameter("xq", [128, NI, S], dt.bfloat16, isOutput=False)
    xk = nc.declare_dram_parameter("xk", [128, NI, S], dt.bfloat16, isOutput=False)
    xv = nc.declare_dram_parameter("xv", [128, NI, S], dt.bfloat16, isOutput=False)
    # wqkv[p, i, :] = (Wq.T | Wk.T | Wv.T)[128*i + p, :] (256 cols each)
    wqkv = nc.declare_dram_parameter("wqkv", [128, NI, 3 * DG], dt.bfloat16, isOutput=False)
    bqkT = nc.declare_dram_parameter("bqkT", [128, 4], dt.float32, isOutput=False)
    # bvb[p, r, :] = bv (same for all p, r): 4x-replicated for batched evictions
    bvb = nc.declare_dram_parameter("bvb", [128, 4, DG], dt.float32, isOutput=False)
    # normalized attention output: [qp*8+qs, p, c] = x[1024qp+128qs+p, c]
    xo = nc.declare_dram_parameter("xo", [16, 128, DG], dt.bfloat16, isOutput=True)

    with tile.TileContext(nc) as tc, ExitStack() as ctx:
        const = ctx.enter_context(tc.tile_pool(name="const", bufs=1))
        biasT = const.tile([128, 4], dt.float32, name="biasT", tag="biasT")
        bvb_sb = const.tile([128, 4, DG], dt.float32, name="bvb", tag="bvb")
        dum = const.tile([128, 512], dt.bfloat16, name="dum", tag="dum")
        nc.vector.memset(dum[:], 0.0)
        nc.sync.dma_start(biasT[:], bqkT[:])
        nc.sync.dma_start(bvb_sb[:], bvb[:])

        xpool = ctx.enter_context(tc.tile_pool(name="x", bufs=1))
        wq_sb = xpool.tile([128, NI, 3 * DG], dt.bfloat16, name="wq_sb", tag="w")
        xk_sb = xpool.tile([128, NI, S], dt.bfloat16, name="xk_sb", tag="xk")
        xq_sb = xpool.tile([128, NI, S], dt.bfloat16, name="xq_sb", tag="xq")
        xv_sb = xpool.tile([128, NI, S], dt.bfloat16, name="xv_sb", tag="xv")

        # staged input DMA: K weights+first keys, Q first window, then K rest
        # just-in-time for the exp pacing, V, and finally the second Q window
        nc.sync.dma_start(wq_sb[:, :, DG:2 * DG], wqkv[:, :, DG:2 * DG])
        nc.sync.dma_start(wq_sb[:, :, 0:DG], wqkv[:, :, 0:DG])
        nc.sync.dma_start(xk_sb[:, :, 0:512], xk[:, :, 0:512])
        nc.sync.dma_start(xq_sb[:, :, 0:512], xq[:, :, 0:512])
        nc.sync.dma_start(xq_sb[:, :, 512:1024], xq[:, :, 512:1024])
        nc.sync.dma_start(xk_sb[:, :, 512:1024], xk[:, :, 512:1024])
        nc.sync.dma_start(xk_sb[:, :, 1024:1536], xk[:, :, 1024:1536])
        nc.sync.dma_start(xk_sb[:, :, 1536:2048], xk[:, :, 1536:2048])
        nc.sync.dma_start(wq_sb[:, :, 2 * DG:3 * DG], wqkv[:, :, 2 * DG:3 * DG])
        nc.sync.dma_start(xv_sb[:, :, 0:QP], xv[:, :, 0:QP])
        nc.sync.dma_start(xv_sb[:, :, QP:S], xv[:, :, QP:S])
        nc.sync.dma_start(xq_sb[:, :, QP:S], xq[:, :, QP:S])

        kT = [xpool.tile([128, S], dt.bfloat16, name=f"kT{p}", tag=f"kT{p}") for p in range(2)]
        qT = [xpool.tile([128, S], dt.bfloat16, name=f"qT{p}", tag=f"qT{p}") for p in range(2)]
        vext = xpool.tile([128, NKS, HL, DH + 1], dt.bfloat16, name="vext", tag="vext")
        nc.vector.memset(vext[:, :, :, DH:DH + 1], 1.0)
        xo_big = [xpool.tile([128, NQS, DG], dt.bfloat16, name=f"xo{qp}", tag=f"xo{qp}")
                  for qp in range(2)]

        pp = ctx.enter_context(tc.tile_pool(name="pp", bufs=2, space="PSUM"))
        avp = ctx.enter_context(tc.tile_pool(name="avp", bufs=1, space="PSUM"))
        pj = ctx.enter_context(tc.tile_pool(name="pj", bufs=2, space="PSUM"))
        exps = ctx.enter_context(tc.tile_pool(name="exps", bufs=NEX))
        ex16s = ctx.enter_context(tc.tile_pool(name="ex16s", bufs=3))
        rcps = ctx.enter_context(tc.tile_pool(name="rcps", bufs=2))

        def gen_warmup(n, lhs_ap):
            # keeps the PE p-state hot; lhs_ap gates the block on a DMA
            for _ in range(n):
                ps = pj.tile([128, 512], dt.float32, name="pjt", tag="pjt")
                nc.tensor.matmul(ps[:], lhsT=lhs_ap, rhs=dum[:],
                                 start=True, stop=True)
                yield None

        def gen_proj_chunk(xt, dst, wcol, bias_col, label, c):
            # dst[:, 512c:512c+512] = (W block).T @ x chunk + bias
            ps = pj.tile([128, 512], dt.float32, name="pjt", tag="pjt")
            for i in range(NI):
                nc.tensor.matmul(
                    ps[:], lhsT=wq_sb[:, i, wcol:wcol + 128],
                    rhs=xt[:, i, 512 * c:512 * (c + 1)],
                    start=(i == 0), stop=(i == NI - 1))
                yield None
            nc.vector.tensor_scalar_add(
                dst[:, 512 * c:512 * (c + 1)], ps[:],
                biasT[:, bias_col:bias_col + 1])
            yield (label, c + 1)

        # per-pass state shared with the deferred tail generators
        tails = {}

        def normalize(av, hl, qp, mix=False):
            for lohi in range(2):
                rcp = rcps.tile([128, 4], dt.float32, name="rcp", tag="rcp")
                nc.vector.reciprocal(rcp[:], av[lohi][:, :, DH:DH + 1])
                for q4 in range(4):
                    qs = 4 * lohi + q4
                    dst = xo_big[qp][:, qs, DH * hl:DH * (hl + 1)]
                    src = av[lohi][:, q4, 0:DH]
                    sc = rcp[:, q4:q4 + 1]
                    if mix and q4 % 2 == 1:
                        nc.scalar.mul(dst, src, sc)
                    else:
                        nc.vector.tensor_scalar_mul(dst, src, sc)
                if hl == HL - 1:
                    # dst AP reordered to (p, qs, c) so the SBUF source AP
                    # keeps its partition dim first (walrus requirement)
                    nc.sync.dma_start(
                        xo[8 * qp + 4 * lohi:8 * qp + 4 * lohi + 4, :, :]
                        .transpose([1, 0, 2]),
                        xo_big[qp][:, 4 * lohi:4 * lohi + 4, :])

        def v_unit4(hl, kb):
            # vext[:, 4kb:4kb+4, hl, 0:64] = head hl V for 4 key tiles.
            # j-outer: one open psum accumulation region per bank at a time.
            ps = pj.tile([128, 512], dt.float32, name="pjt", tag="pjt")
            for j in range(4):
                ks = 4 * kb + j
                for i in range(NI):
                    nc.tensor.matmul(
                        ps[:, DH * j:DH * (j + 1)],
                        lhsT=xv_sb[:, i, 128 * ks:128 * (ks + 1)],
                        rhs=wq_sb[:, i, 2 * DG + DH * hl:2 * DG + DH * (hl + 1)],
                        start=(i == 0), stop=(i == NI - 1))
            nc.vector.tensor_tensor(
                vext[:, 4 * kb:4 * kb + 4, hl, 0:DH], ps[:, 0:4 * DH],
                bvb_sb[:, :, DH * hl:DH * (hl + 1)], ALU.add)

        def gen_vh(hl):
            for kb in range(4):
                v_unit4(hl, kb)
                yield (f"vh{hl}", 4 * (kb + 1))

        def gen_tailk(k):
            # pass k's AV sweep + softmax normalize, absorbed into pass k+1.
            # q4-outer so each psum bank has ONE open accumulation region at a
            # time (hardware requirement); the two av banks run in parallel.
            # Progress label t{k}: count 16*q4 + ks + 1; ex[ks] is free once
            # count >= 48 + ks + 1 (its q4=3 read).
            if k == 0:
                for kb in range(4):
                    v_unit4(0, kb)
                    yield ("vh0", 4 * (kb + 1))
            av, exs, hl, qp = tails[k]
            for q4 in range(4):
                for ks in range(NKS):
                    nc.tensor.matmul(
                        av[0][:, q4, :],
                        lhsT=exs[ks][:, 128 * q4:128 * (q4 + 1)],
                        rhs=vext[:, ks, hl, :],
                        start=(ks == 0), stop=(ks == NKS - 1))
                    nc.tensor.matmul(
                        av[1][:, q4, :],
                        lhsT=exs[ks][:, 128 * (q4 + 4):128 * (q4 + 5)],
                        rhs=vext[:, ks, hl, :],
                        start=(ks == 0), stop=(ks == NKS - 1))
                    yield (f"t{k}", 16 * q4 + ks + 1)
            normalize(av, hl, qp, mix=(k == 7))
            yield (f"t{k}", 65)

        class Filler:
            def __init__(self, units):
                self.units = units
                self.done = {}
                self.empty = False
                self.gate = None       # blocked on this closed gate
                self.open = set()

            def pull(self, n=1):
                for _ in range(n):
                    if self.gate is not None:
                        if self.gate not in self.open:
                            return
                        self.gate = None
                    try:
                        lab = next(self.units)
                    except StopIteration:
                        self.empty = True
                        return
                    if lab is None:
                        continue
                    if lab[0] == "GATE":
                        if lab[1] not in self.open:
                            self.gate = lab[1]
                            return
                    else:
                        self.done[lab[0]] = lab[1]

            def open_gate(self, k):
                self.open.add(k)

            def require(self, key, count):
                while self.done.get(key, 0) < count:
                    assert not self.empty, f"filler exhausted before {key}={count}"
                    assert not (self.gate is not None and self.gate not in self.open), \
                        f"require {key}={count} blocked by gate {self.gate}"
                    self.pull(1)

        def gen_gate(k):
            yield ("GATE", k)

        def k_chunk(p, c):
            return gen_proj_chunk(xk_sb, kT[p], DG + 128 * p, 2 + p, f"k{p}", c)

        def q_chunk(p, c):
            return gen_proj_chunk(xq_sb, qT[p], 128 * p, p, f"q{p}", c)

        fill = Filler(chain(
            gen_warmup(10, dum[:, 0:128]),
            gen_warmup(15, wq_sb[:, 0, DG:DG + 128]),
            k_chunk(0, 0), q_chunk(0, 0), q_chunk(0, 1),
            k_chunk(0, 1), k_chunk(0, 2), k_chunk(0, 3),
            k_chunk(1, 0), k_chunk(1, 1), k_chunk(1, 2), k_chunk(1, 3),
            gen_gate(0), gen_tailk(0),
            gen_vh(1), q_chunk(1, 0), q_chunk(1, 1),
            gen_gate(1), gen_tailk(1),
            gen_vh(2), q_chunk(0, 2), q_chunk(0, 3),
            gen_gate(2), gen_tailk(2),
            gen_vh(3), q_chunk(1, 2), q_chunk(1, 3),
            gen_gate(3), gen_tailk(3),
            gen_gate(4), gen_tailk(4),
            gen_gate(5), gen_tailk(5),
            gen_gate(6), gen_tailk(6),
            gen_gate(7), gen_tailk(7),
        ))

        def emit_exp(qk, dve):
            if dve:
                exi = ex16s.tile([128, QP], dt.int16, name="exi", tag="exi")
                nc.vector.tensor_scalar(
                    exi[:], qk[:], A_DVE, B_DVE, ALU.mult, ALU.add)
                return exi.bitcast(dt.bfloat16)
            ext = exps.tile([128, QP], dt.bfloat16, name="ex", tag="ex")
            nc.scalar.activation(ext[:], qk[:], AF.Exp, scale=0.125)
            return ext

        def emit_qk(p, lo, qp, ks):
            qk = pp.tile([128, QP], dt.float32, name="qkt", tag="qkt")
            for nh in range(2):
                nc.tensor.matmul(
                    qk[:, 512 * nh:512 * (nh + 1)],
                    lhsT=kT[p][lo:lo + 64, 128 * ks:128 * (ks + 1)],
                    rhs=qT[p][lo:lo + 64, QP * qp + 512 * nh:QP * qp + 512 * nh + 512],
                    start=True, stop=True)
            return qk

        def attention_pass(idx, hl, qp):
            p, half = divmod(hl, 2)
            lo = 64 * half
            fill.require(f"q{p}", 2 * (qp + 1))
            av_lo = avp.tile([128, 4, DH + 1], dt.float32, name="av_lo", tag="av_lo")
            av_hi = avp.tile([128, 4, DH + 1], dt.float32, name="av_hi", tag="av_hi")
            exs = []
            tails[idx] = ([av_lo, av_hi], exs, hl, qp)
            for ks in range(NKS):
                fill.require(f"k{p}", min(ks // 4 + 1, 4))
                n = 16 * idx + ks
                if n - NEX >= 0:
                    ep, eks = divmod(n - NEX, NKS)
                    fill.require(f"t{ep}", 48 + eks + 1)
                qk = emit_qk(p, lo, qp, ks)
                exs.append(emit_exp(qk, idx in ALT_PASSES and ks % 2 == 1))
                fill.pull(5 if idx == 0 and ks >= 6 else 4)
            fill.open_gate(idx)

        order = [(0, 0), (1, 0), (2, 0), (3, 0), (0, 1), (1, 1), (2, 1), (3, 1)]
        for idx, (hl, qp) in enumerate(order):
            attention_pass(idx, hl, qp)
        while not fill.empty:
            fill.pull(4)

    nc.compile()
    return nc


def build_stage2(n_cores=8):
    nc = bacc.Bacc("TRN2", target_bir_lowering=False, debug=False, num_devices=n_cores)
    # xT[p, i, r] = x^T[128i + p, r] for this core's CHUNK rows
    xT = nc.declare_dram_parameter("xT", [128, NI, CHUNK], dt.bfloat16, isOutput=False)
    # woT[p, i, c] = Wo.T[128i + p, c]
    woT = nc.declare_dram_parameter("woT", [128, NI, D], dt.bfloat16, isOutput=False)
    bo = nc.declare_dram_parameter("bo", [1, D], dt.bfloat16, isOutput=False)
    out = nc.declare_dram_parameter("out", [CHUNK, D], dt.float32, isOutput=True)

    with tile.TileContext(nc) as tc, ExitStack() as ctx:
        pool = ctx.enter_context(tc.tile_pool(name="p2", bufs=1))
        ones_k1 = pool.tile([1, 512], dt.bfloat16, name="ones_k1", tag="ones")
        nc.vector.memset(ones_k1[:], 1.0)
        bo_sb = pool.tile([1, D], dt.bfloat16, name="bo_sb", tag="bo")
        nc.sync.dma_start(bo_sb[:], bo[:])
        xT_sb = pool.tile([128, NI, CHUNK], dt.bfloat16, name="xT_sb", tag="xT")
        wo_sb = pool.tile([128, NI, D], dt.bfloat16, name="wo_sb", tag="wo")
        for c in range(4):
            nc.sync.dma_start(xT_sb[:, 2 * c:2 * c + 2, :], xT[:, 2 * c:2 * c + 2, :])
            nc.sync.dma_start(wo_sb[:, 2 * c:2 * c + 2, :], woT[:, 2 * c:2 * c + 2, :])

        psp = ctx.enter_context(tc.tile_pool(name="psp", bufs=5, space="PSUM"))
        # PE p-state warmup while the first DMA chunks land
        for _ in range(10):
            ps = psp.tile([128, 512], dt.float32, name="fc", tag="fc")
            nc.tensor.matmul(ps[:], lhsT=ones_k1[0:1, 0:128], rhs=ones_k1[:],
                             start=True, stop=True)

        def mm(ps, g, t):
            rt, nh = divmod(g, 2)
            nc.tensor.matmul(
                ps[:], lhsT=xT_sb[:, t, 128 * rt:128 * (rt + 1)],
                rhs=wo_sb[:, t, 512 * nh:512 * (nh + 1)],
                start=(t == 0), stop=False)

        def finish(ps, g):
            rt, nh = divmod(g, 2)
            nc.tensor.matmul(
                ps[:], lhsT=ones_k1[0:1, 0:128],
                rhs=bo_sb[0:1, 512 * nh:512 * (nh + 1)],
                start=False, stop=True)
            os = pool.tile([128, 512], dt.float32, name=f"os{g}", tag=f"os{g % 4}",
                           bufs=1)
            if g % 2 == 0:
                nc.scalar.copy(os[:], ps[:])
            else:
                nc.vector.tensor_copy(os[:], ps[:])
            nc.sync.dma_start(
                out[128 * rt:128 * (rt + 1), 512 * nh:512 * (nh + 1)], os[:])

        # groups 0-3 interleaved across t (PE keeps pace with the input DMA
        # window and they finish as it ends), then groups 4..7 group-outer so
        # output DMA and evictions overlap the remaining compute
        ps4 = [psp.tile([128, 512], dt.float32, name="fc", tag="fc")
               for _ in range(4)]
        for t in range(NI):
            for g in range(4):
                mm(ps4[g], g, t)
        for g in range(4):
            finish(ps4[g], g)
        for g in range(4, 8):
            ps = psp.tile([128, 512], dt.float32, name="fc", tag="fc")
            for t in range(NI):
                mm(ps, g, t)
            finish(ps, g)

    nc.compile()
    return nc


def _fold(a):
    """[Dfull, N] -> [128, Dfull//128, N] with element (p, i, n) = a[128i+p, n]."""
    d, n = a.shape
    return np.ascontiguousarray(a.reshape(d // 128, 128, n).transpose(1, 0, 2))


def stage1_inputs(inputs):
    """core c = (b, g): b = c // 4, g = c % 4 (head-group of 4 heads)."""
    xt = {}
    for nm in ('query', 'key', 'value'):
        for b in range(B):
            xt[(nm, b)] = _fold(np.ascontiguousarray(
                np.asarray(inputs[nm])[b].T).astype(BF16))
    maps = []
    for c in range(8):
        b, g = divmod(c, 4)
        rows = slice(DG * g, DG * (g + 1))
        bq = np.asarray(inputs['bq'])[rows].astype(np.float32)
        bk = np.asarray(inputs['bk'])[rows].astype(np.float32)
        bv = np.asarray(inputs['bv'])[rows].astype(np.float32)
        bqkT = np.stack([bq[0:128], bq[128:256], bk[0:128], bk[128:256]], axis=1)
        bvb = np.broadcast_to(bv[None, None, :], (128, 4, DG)).copy()
        w = np.concatenate([
            np.asarray(inputs['Wq'])[rows].T,
            np.asarray(inputs['Wk'])[rows].T,
            np.asarray(inputs['Wv'])[rows].T], axis=1).astype(BF16)
        maps.append({
            'xq': xt[('query', b)], 'xk': xt[('key', b)], 'xv': xt[('value', b)],
            'wqkv': _fold(np.ascontiguousarray(w)),
            'bqkT': bqkT, 'bvb': bvb,
        })
    return maps


def stage2_inputs(stage1_results, inputs):
    """stage1_results: list of 8 dicts with 'xo' [16, 128, 256] bf16 (normalized)."""
    woT = _fold(np.ascontiguousarray(np.asarray(inputs['Wo']).T).astype(BF16))
    bo = np.asarray(inputs['bo'])[None, :].astype(BF16)
    xrows_all = {}
    for cc in range(8):
        xrows_all[cc] = np.asarray(stage1_results[cc]['xo']).reshape(S, DG)
    maps = []
    for c in range(8):
        b, j = divmod(c, 4)
        rows = slice(CHUNK * j, CHUNK * (j + 1))
        xrows = np.concatenate(
            [xrows_all[4 * b + g][rows] for g in range(4)], axis=1)  # [512, 1024]
        maps.append({
            'xT': _fold(np.ascontiguousarray(xrows.T)),
            'woT': woT, 'bo': bo,
        })
    return maps


def assemble_output(stage2_results):
    out = np.empty((B, S, D), dtype=np.float32)
    for c in range(8):
        b, j = divmod(c, 4)
        out[b, CHUNK * j:CHUNK * (j + 1), :] = np.asarray(stage2_results[c]['out'])
    return out


_CACHE = {}


def _programs():
    if 'nc1' not in _CACHE:
        _CACHE['nc1'] = build_stage1(N_CORES)
        _CACHE['nc2'] = build_stage2(N_CORES)
    return _CACHE['nc1'], _CACHE['nc2']


def kernel(**inputs):
    nc1, nc2 = _programs()
    core_ids = list(range(N_CORES))
    s1_maps = stage1_inputs(inputs)
    r1 = run_bass_kernel_spmd(nc1, s1_maps, core_ids).results
    s2_maps = stage2_inputs(r1, inputs)
    r2 = run_bass_kernel_spmd(nc2, s2_maps, core_ids).results
    return assemble_output(r2)


# revision 31
# speedup vs baseline: 1.0435x; 1.0287x over previous
"""Self-contained Trainium2 Bass kernel for nn_MultiHeadAttention_46969762349562.

Full fp32 inputs -> full fp32 output, distributed over 8 NeuronCores in two
SPMD stages (all FLOPs on device; host only slices/casts/concats/transposes):

  stage 1 (core = batch x 4-head group): Q/K/V projections (column-parallel
    weights, activations staged pre-transposed in bf16), attention with
    transposed score tiles [k_seq, q].  AV uses the transposed formulation
    out[q, vdim] = ex^T @ [V | ones]  (ex as the matmul stationary): 65-col
    AV matmuls at full PE utilization, half the PE cycles of the moving-ex
    form, with the softmax denominator free in column 64.  Scores are O(+-6)
    so no max-subtraction is needed.  exp runs on the ACT engine.  Because
    PSUM allows only ONE open accumulation region per bank, each pass's AV
    sweep (q4-outer over its two av banks) plus softmax normalization (on
    DVE; the denominator is a per-partition scalar in this layout) is
    deferred into the NEXT pass's timeline via gated tail generators; the
    exp tiles live in a 28-slot ring whose slot reuse is sequenced by tail
    progress labels.  Projections and per-head V are software-pipelined the
    same way with just-in-time requirements matched to the DMA arrival
    order; warmup matmuls keep the PE p-state hot through the DMA-bound
    startup.
  host reshard: head-sharded row-major x -> row-sharded x^T (slices+transpose).
  stage 2 (core = batch x 512-row chunk): output projection + bias only,
    group-outer accumulation so output DMA overlaps compute.
"""
import sys
for p in ('/opt/trn_rl_repo', '/root/.axon_site/_ro/trn_rl_repo'):
    if p not in sys.path:
        sys.path.append(p)
import numpy as np
import ml_dtypes
import concourse.bass as bass
import concourse.bacc as bacc
import concourse.mybir as mybir
from concourse import tile
from concourse.bass_utils import run_bass_kernel_spmd
from contextlib import ExitStack
from itertools import chain

dt = mybir.dt
AF = mybir.ActivationFunctionType
ALU = mybir.AluOpType
BF16 = ml_dtypes.bfloat16

B = 2
S = 2048
D = 1024
H = 16
DH = 64
HL = 4            # heads per core (stage 1)
DG = HL * DH      # 256 dims per head-group
NI = D // 128     # 8 contraction tiles
NKS = S // 128    # 16 key tiles
QP = 1024         # q-pass width
NQS = QP // 128   # 8 q sub-blocks per pass
CHUNK = S // 4    # 512 rows per stage-2 core
N_CORES = 8
NEX = 28          # exp-tile ring size (2 banks hold one open
                  # accumulation region each, so AV defers a full pass)

# Schraudolph exp on DVE: bf16bits(exp(x/8)) ~= int16(qk * A_DVE + B_DVE)
A_DVE = 128.0 / np.log(2.0) / 8.0
B_DVE = 16248.6               # 127*128 recentred for the sawtooth bias
ALT_PASSES = (4, 5, 6, 7)  # late passes alternate exp between ACT and DVE


def build_stage1(n_cores=8):
    nc = bacc.Bacc("TRN2", target_bir_lowering=False, debug=False, num_devices=n_cores)
    # [p, i, s] layouts: element (p, i, s) = x^T[128*i + p, s]
    xq = nc.declare_dram_parameter("xq", [128, NI, S], dt.bfloat16, isOutput=False)
    xk = nc.declare_dram_parameter("xk", [128, NI, S], dt.bfloat16, isOutput=False)
    xv = nc.declare_dram_parameter("xv", [128, NI, S], dt.bfloat16, isOutput=False)
    # wqkv[p, i, :] = (Wq.T | Wk.T | Wv.T)[128*i + p, :] (256 cols each)
    wqkv = nc.declare_dram_parameter("wqkv", [128, NI, 3 * DG], dt.bfloat16, isOutput=False)
    bqkT = nc.declare_dram_parameter("bqkT", [128, 4], dt.float32, isOutput=False)
    # bvb[p, r, :] = bv (same for all p, r): 4x-replicated for batched evictions
    bvb = nc.declare_dram_parameter("bvb", [128, 4, DG], dt.float32, isOutput=False)
    # normalized attention output: [qp*8+qs, p, c] = x[1024qp+128qs+p, c]
    xo = nc.declare_dram_parameter("xo", [16, 128, DG], dt.bfloat16, isOutput=True)

    with tile.TileContext(nc) as tc, ExitStack() as ctx:
        const = ctx.enter_context(tc.tile_pool(name="const", bufs=1))
        biasT = const.tile([128, 4], dt.float32, name="biasT", tag="biasT")
        bvb_sb = const.tile([128, 4, DG], dt.float32, name="bvb", tag="bvb")
        dum = const.tile([128, 512], dt.bfloat16, name="dum", tag="dum")
        nc.vector.memset(dum[:], 0.0)
        nc.sync.dma_start(biasT[:], bqkT[:])
        nc.sync.dma_start(bvb_sb[:], bvb[:])

        xpool = ctx.enter_context(tc.tile_pool(name="x", bufs=1))
        wq_sb = xpool.tile([128, NI, 3 * DG], dt.bfloat16, name="wq_sb", tag="w")
        xk_sb = xpool.tile([128, NI, S], dt.bfloat16, name="xk_sb", tag="xk")
        xq_sb = xpool.tile([128, NI, S], dt.bfloat16, name="xq_sb", tag="xq")
        xv_sb = xpool.tile([128, NI, S], dt.bfloat16, name="xv_sb", tag="xv")

        # staged input DMA: K weights+first keys, Q first window, then K rest
        # just-in-time for the exp pacing, V, and finally the second Q window
        nc.sync.dma_start(wq_sb[:, :, DG:2 * DG], wqkv[:, :, DG:2 * DG])
        nc.sync.dma_start(wq_sb[:, :, 0:DG], wqkv[:, :, 0:DG])
        nc.sync.dma_start(xk_sb[:, :, 0:512], xk[:, :, 0:512])
        nc.sync.dma_start(xq_sb[:, :, 0:512], xq[:, :, 0:512])
        nc.sync.dma_start(xq_sb[:, :, 512:1024], xq[:, :, 512:1024])
        nc.sync.dma_start(xk_sb[:, :, 512:1024], xk[:, :, 512:1024])
        nc.sync.dma_start(xk_sb[:, :, 1024:1536], xk[:, :, 1024:1536])
        nc.sync.dma_start(xk_sb[:, :, 1536:2048], xk[:, :, 1536:2048])
        nc.sync.dma_start(wq_sb[:, :, 2 * DG:3 * DG], wqkv[:, :, 2 * DG:3 * DG])
        nc.sync.dma_start(xv_sb[:, :, 0:QP], xv[:, :, 0:QP])
        nc.sync.dma_start(xv_sb[:, :, QP:S], xv[:, :, QP:S])
        nc.sync.dma_start(xq_sb[:, :, QP:S], xq[:, :, QP:S])

        kT = [xpool.tile([128, S], dt.bfloat16, name=f"kT{p}", tag=f"kT{p}") for p in range(2)]
        qT = [xpool.tile([128, S], dt.bfloat16, name=f"qT{p}", tag=f"qT{p}") for p in range(2)]
        vext = xpool.tile([128, NKS, HL, DH + 1], dt.bfloat16, name="vext", tag="vext")
        nc.vector.memset(vext[:, :, :, DH:DH + 1], 1.0)
        xo_big = [xpool.tile([128, NQS, DG], dt.bfloat16, name=f"xo{qp}", tag=f"xo{qp}")
                  for qp in range(2)]

        pp = ctx.enter_context(tc.tile_pool(name="pp", bufs=2, space="PSUM"))
        avp = ctx.enter_context(tc.tile_pool(name="avp", bufs=1, space="PSUM"))
        pj = ctx.enter_context(tc.tile_pool(name="pj", bufs=2, space="PSUM"))
        exps = ctx.enter_context(tc.tile_pool(name="exps", bufs=NEX))
        ex16s = ctx.enter_context(tc.tile_pool(name="ex16s", bufs=3))
        rcps = ctx.enter_context(tc.tile_pool(name="rcps", bufs=2))

        def gen_warmup(n, lhs_ap):
            # keeps the PE p-state hot; lhs_ap gates the block on a DMA
            for _ in range(n):
                ps = pj.tile([128, 512], dt.float32, name="pjt", tag="pjt")
                nc.tensor.matmul(ps[:], lhsT=lhs_ap, rhs=dum[:],
                                 start=True, stop=True)
                yield None

        def gen_proj_chunk(xt, dst, wcol, bias_col, label, c):
            # dst[:, 512c:512c+512] = (W block).T @ x chunk + bias
            ps = pj.tile([128, 512], dt.float32, name="pjt", tag="pjt")
            for i in range(NI):
                nc.tensor.matmul(
                    ps[:], lhsT=wq_sb[:, i, wcol:wcol + 128],
                    rhs=xt[:, i, 512 * c:512 * (c + 1)],
                    start=(i == 0), stop=(i == NI - 1))
                yield None
            nc.vector.tensor_scalar_add(
                dst[:, 512 * c:512 * (c + 1)], ps[:],
                biasT[:, bias_col:bias_col + 1])
            yield (label, c + 1)

        # per-pass state shared with the deferred tail generators
        tails = {}

        def normalize(av, hl, qp, mix=False):
            for lohi in range(2):
                rcp = rcps.tile([128, 4], dt.float32, name="rcp", tag="rcp")
                nc.vector.reciprocal(rcp[:], av[lohi][:, :, DH:DH + 1])
                for q4 in range(4):
                    qs = 4 * lohi + q4
                    dst = xo_big[qp][:, qs, DH * hl:DH * (hl + 1)]
                    src = av[lohi][:, q4, 0:DH]
                    sc = rcp[:, q4:q4 + 1]
                    if mix and q4 % 2 == 1:
                        nc.scalar.mul(dst, src, sc)
                    else:
                        nc.vector.tensor_scalar_mul(dst, src, sc)
                if hl == HL - 1:
                    # dst AP reordered to (p, qs, c) so the SBUF source AP
                    # keeps its partition dim first (walrus requirement)
                    nc.sync.dma_start(
                        xo[8 * qp + 4 * lohi:8 * qp + 4 * lohi + 4, :, :]
                        .transpose([1, 0, 2]),
                        xo_big[qp][:, 4 * lohi:4 * lohi + 4, :])

        def v_unit4(hl, kb):
            # vext[:, 4kb:4kb+4, hl, 0:64] = head hl V for 4 key tiles.
            # j-outer: one open psum accumulation region per bank at a time.
            ps = pj.tile([128, 512], dt.float32, name="pjt", tag="pjt")
            for j in range(4):
                ks = 4 * kb + j
                for i in range(NI):
                    nc.tensor.matmul(
                        ps[:, DH * j:DH * (j + 1)],
                        lhsT=xv_sb[:, i, 128 * ks:128 * (ks + 1)],
                        rhs=wq_sb[:, i, 2 * DG + DH * hl:2 * DG + DH * (hl + 1)],
                        start=(i == 0), stop=(i == NI - 1))
            nc.vector.tensor_tensor(
                vext[:, 4 * kb:4 * kb + 4, hl, 0:DH], ps[:, 0:4 * DH],
                bvb_sb[:, :, DH * hl:DH * (hl + 1)], ALU.add)

        def gen_vh(hl):
            for kb in range(4):
                v_unit4(hl, kb)
                yield (f"vh{hl}", 4 * (kb + 1))

        def gen_tailk(k):
            # pass k's AV sweep + softmax normalize, absorbed into pass k+1.
            # q4-outer so each psum bank has ONE open accumulation region at a
            # time (hardware requirement); the two av banks run in parallel.
            # Progress label t{k}: count 16*q4 + ks + 1; ex[ks] is free once
            # count >= 48 + ks + 1 (its q4=3 read).
            if k == 0:
                for kb in range(4):
                    v_unit4(0, kb)
                    yield ("vh0", 4 * (kb + 1))
            av, exs, hl, qp = tails[k]
            for q4 in range(4):
                for ks in range(NKS):
                    nc.tensor.matmul(
                        av[0][:, q4, :],
                        lhsT=exs[ks][:, 128 * q4:128 * (q4 + 1)],
                        rhs=vext[:, ks, hl, :],
                        start=(ks == 0), stop=(ks == NKS - 1))
                    nc.tensor.matmul(
                        av[1][:, q4, :],
                        lhsT=exs[ks][:, 128 * (q4 + 4):128 * (q4 + 5)],
                        rhs=vext[:, ks, hl, :],
                        start=(ks == 0), stop=(ks == NKS - 1))
                    yield (f"t{k}", 16 * q4 + ks + 1)
            normalize(av, hl, qp, mix=(k == 7))
            yield (f"t{k}", 65)

        class Filler:
            def __init__(self, units):
                self.units = units
                self.done = {}
                self.empty = False
                self.gate = None       # blocked on this closed gate
                self.open = set()

            def pull(self, n=1):
                for _ in range(n):
                    if self.gate is not None:
                        if self.gate not in self.open:
                            return
                        self.gate = None
                    try:
                        lab = next(self.units)
                    except StopIteration:
                        self.empty = True
                        return
                    if lab is None:
                        continue
                    if lab[0] == "GATE":
                        if lab[1] not in self.open:
                            self.gate = lab[1]
                            return
                    else:
                        self.done[lab[0]] = lab[1]

            def open_gate(self, k):
                self.open.add(k)

            def require(self, key, count):
                while self.done.get(key, 0) < count:
                    assert not self.empty, f"filler exhausted before {key}={count}"
                    assert not (self.gate is not None and self.gate not in self.open), \
                        f"require {key}={count} blocked by gate {self.gate}"
                    self.pull(1)

        def gen_gate(k):
            yield ("GATE", k)

        def k_chunk(p, c):
            return gen_proj_chunk(xk_sb, kT[p], DG + 128 * p, 2 + p, f"k{p}", c)

        def q_chunk(p, c):
            return gen_proj_chunk(xq_sb, qT[p], 128 * p, p, f"q{p}", c)

        fill = Filler(chain(
            gen_warmup(10, dum[:, 0:128]),
            gen_warmup(15, wq_sb[:, 0, DG:DG + 128]),
            k_chunk(0, 0), q_chunk(0, 0), q_chunk(0, 1),
            k_chunk(0, 1), k_chunk(0, 2), k_chunk(0, 3),
            k_chunk(1, 0), k_chunk(1, 1), k_chunk(1, 2), k_chunk(1, 3),
            gen_gate(0), gen_tailk(0),
            gen_vh(1), q_chunk(1, 0), q_chunk(1, 1),
            gen_gate(1), gen_tailk(1),
            gen_vh(2), q_chunk(0, 2), q_chunk(0, 3),
            gen_gate(2), gen_tailk(2),
            gen_vh(3), q_chunk(1, 2), q_chunk(1, 3),
            gen_gate(3), gen_tailk(3),
            gen_gate(4), gen_tailk(4),
            gen_gate(5), gen_tailk(5),
            gen_gate(6), gen_tailk(6),
            gen_gate(7), gen_tailk(7),
        ))

        def emit_exp(qk, dve):
            if dve:
                # Schraudolph tile lives in the same ring (same tag/slot size)
                exi = exps.tile([128, QP], dt.int16, name="exi", tag="ex")
                nc.vector.tensor_scalar(
                    exi[:], qk[:], A_DVE, B_DVE, ALU.mult, ALU.add)
                return exi.bitcast(dt.bfloat16)
            ext = exps.tile([128, QP], dt.bfloat16, name="ex", tag="ex")
            nc.scalar.activation(ext[:], qk[:], AF.Exp, scale=0.125)
            return ext

        def emit_qk(p, lo, qp, ks):
            qk = pp.tile([128, QP], dt.float32, name="qkt", tag="qkt")
            for nh in range(2):
                nc.tensor.matmul(
                    qk[:, 512 * nh:512 * (nh + 1)],
                    lhsT=kT[p][lo:lo + 64, 128 * ks:128 * (ks + 1)],
                    rhs=qT[p][lo:lo + 64, QP * qp + 512 * nh:QP * qp + 512 * nh + 512],
                    start=True, stop=True)
            return qk

        def attention_pass(idx, hl, qp):
            p, half = divmod(hl, 2)
            lo = 64 * half
            fill.require(f"q{p}", 2 * (qp + 1))
            av_lo = avp.tile([128, 4, DH + 1], dt.float32, name="av_lo", tag="av_lo")
            av_hi = avp.tile([128, 4, DH + 1], dt.float32, name="av_hi", tag="av_hi")
            exs = []
            tails[idx] = ([av_lo, av_hi], exs, hl, qp)
            for ks in range(NKS):
                fill.require(f"k{p}", min(ks // 4 + 1, 4))
                n = 16 * idx + ks
                if n - NEX >= 0:
                    ep, eks = divmod(n - NEX, NKS)
                    fill.require(f"t{ep}", 48 + eks + 1)
                qk = emit_qk(p, lo, qp, ks)
                exs.append(emit_exp(qk, idx in ALT_PASSES and ks % 2 == 1))
                fill.pull(5 if idx == 0 and ks >= 6 else 4)
            fill.open_gate(idx)

        order = [(0, 0), (1, 0), (2, 0), (3, 0), (0, 1), (1, 1), (2, 1), (3, 1)]
        for idx, (hl, qp) in enumerate(order):
            attention_pass(idx, hl, qp)
        while not fill.empty:
            fill.pull(4)

    nc.compile()
    return nc


def build_stage2(n_cores=8):
    nc = bacc.Bacc("TRN2", target_bir_lowering=False, debug=False, num_devices=n_cores)
    # xT[p, i, r] = x^T[128i + p, r] for this core's CHUNK rows
    xT = nc.declare_dram_parameter("xT", [128, NI, CHUNK], dt.bfloat16, isOutput=False)
    # woT[p, i, c] = Wo.T[128i + p, c]
    woT = nc.declare_dram_parameter("woT", [128, NI, D], dt.bfloat16, isOutput=False)
    bo = nc.declare_dram_parameter("bo", [1, D], dt.bfloat16, isOutput=False)
    out = nc.declare_dram_parameter("out", [CHUNK, D], dt.float32, isOutput=True)

    with tile.TileContext(nc) as tc, ExitStack() as ctx:
        pool = ctx.enter_context(tc.tile_pool(name="p2", bufs=1))
        ones_k1 = pool.tile([1, 512], dt.bfloat16, name="ones_k1", tag="ones")
        nc.vector.memset(ones_k1[:], 1.0)
        bo_sb = pool.tile([1, D], dt.bfloat16, name="bo_sb", tag="bo")
        nc.sync.dma_start(bo_sb[:], bo[:])
        xT_sb = pool.tile([128, NI, CHUNK], dt.bfloat16, name="xT_sb", tag="xT")
        wo_sb = pool.tile([128, NI, D], dt.bfloat16, name="wo_sb", tag="wo")
        for c in range(4):
            nc.sync.dma_start(xT_sb[:, 2 * c:2 * c + 2, :], xT[:, 2 * c:2 * c + 2, :])
            nc.sync.dma_start(wo_sb[:, 2 * c:2 * c + 2, :], woT[:, 2 * c:2 * c + 2, :])

        psp = ctx.enter_context(tc.tile_pool(name="psp", bufs=5, space="PSUM"))
        # PE p-state warmup while the first DMA chunks land
        for _ in range(10):
            ps = psp.tile([128, 512], dt.float32, name="fc", tag="fc")
            nc.tensor.matmul(ps[:], lhsT=ones_k1[0:1, 0:128], rhs=ones_k1[:],
                             start=True, stop=True)

        def mm(ps, g, t):
            rt, nh = divmod(g, 2)
            nc.tensor.matmul(
                ps[:], lhsT=xT_sb[:, t, 128 * rt:128 * (rt + 1)],
                rhs=wo_sb[:, t, 512 * nh:512 * (nh + 1)],
                start=(t == 0), stop=False)

        def finish(ps, g):
            rt, nh = divmod(g, 2)
            nc.tensor.matmul(
                ps[:], lhsT=ones_k1[0:1, 0:128],
                rhs=bo_sb[0:1, 512 * nh:512 * (nh + 1)],
                start=False, stop=True)
            os = pool.tile([128, 512], dt.float32, name=f"os{g}", tag=f"os{g % 4}",
                           bufs=1)
            if g % 2 == 0:
                nc.scalar.copy(os[:], ps[:])
            else:
                nc.vector.tensor_copy(os[:], ps[:])
            nc.sync.dma_start(
                out[128 * rt:128 * (rt + 1), 512 * nh:512 * (nh + 1)], os[:])

        # groups 0-3 interleaved across t (PE keeps pace with the input DMA
        # window and they finish as it ends), then groups 4..7 group-outer so
        # output DMA and evictions overlap the remaining compute
        ps4 = [psp.tile([128, 512], dt.float32, name="fc", tag="fc")
               for _ in range(4)]
        for t in range(NI):
            for g in range(4):
                mm(ps4[g], g, t)
        for g in range(4):
            finish(ps4[g], g)
        for g in range(4, 8):
            ps = psp.tile([128, 512], dt.float32, name="fc", tag="fc")
            for t in range(NI):
                mm(ps, g, t)
            finish(ps, g)

    nc.compile()
    return nc


def _fold(a):
    """[Dfull, N] -> [128, Dfull//128, N] with element (p, i, n) = a[128i+p, n]."""
    d, n = a.shape
    return np.ascontiguousarray(a.reshape(d // 128, 128, n).transpose(1, 0, 2))


def stage1_inputs(inputs):
    """core c = (b, g): b = c // 4, g = c % 4 (head-group of 4 heads)."""
    xt = {}
    for nm in ('query', 'key', 'value'):
        for b in range(B):
            xt[(nm, b)] = _fold(np.ascontiguousarray(
                np.asarray(inputs[nm])[b].T).astype(BF16))
    maps = []
    for c in range(8):
        b, g = divmod(c, 4)
        rows = slice(DG * g, DG * (g + 1))
        bq = np.asarray(inputs['bq'])[rows].astype(np.float32)
        bk = np.asarray(inputs['bk'])[rows].astype(np.float32)
        bv = np.asarray(inputs['bv'])[rows].astype(np.float32)
        bqkT = np.stack([bq[0:128], bq[128:256], bk[0:128], bk[128:256]], axis=1)
        bvb = np.broadcast_to(bv[None, None, :], (128, 4, DG)).copy()
        w = np.concatenate([
            np.asarray(inputs['Wq'])[rows].T,
            np.asarray(inputs['Wk'])[rows].T,
            np.asarray(inputs['Wv'])[rows].T], axis=1).astype(BF16)
        maps.append({
            'xq': xt[('query', b)], 'xk': xt[('key', b)], 'xv': xt[('value', b)],
            'wqkv': _fold(np.ascontiguousarray(w)),
            'bqkT': bqkT, 'bvb': bvb,
        })
    return maps


def stage2_inputs(stage1_results, inputs):
    """stage1_results: list of 8 dicts with 'xo' [16, 128, 256] bf16 (normalized)."""
    woT = _fold(np.ascontiguousarray(np.asarray(inputs['Wo']).T).astype(BF16))
    bo = np.asarray(inputs['bo'])[None, :].astype(BF16)
    xrows_all = {}
    for cc in range(8):
        xrows_all[cc] = np.asarray(stage1_results[cc]['xo']).reshape(S, DG)
    maps = []
    for c in range(8):
        b, j = divmod(c, 4)
        rows = slice(CHUNK * j, CHUNK * (j + 1))
        xrows = np.concatenate(
            [xrows_all[4 * b + g][rows] for g in range(4)], axis=1)  # [512, 1024]
        maps.append({
            'xT': _fold(np.ascontiguousarray(xrows.T)),
            'woT': woT, 'bo': bo,
        })
    return maps


def assemble_output(stage2_results):
    out = np.empty((B, S, D), dtype=np.float32)
    for c in range(8):
        b, j = divmod(c, 4)
        out[b, CHUNK * j:CHUNK * (j + 1), :] = np.asarray(stage2_results[c]['out'])
    return out


_CACHE = {}


def _programs():
    if 'nc1' not in _CACHE:
        _CACHE['nc1'] = build_stage1(N_CORES)
        _CACHE['nc2'] = build_stage2(N_CORES)
    return _CACHE['nc1'], _CACHE['nc2']


def kernel(**inputs):
    nc1, nc2 = _programs()
    core_ids = list(range(N_CORES))
    s1_maps = stage1_inputs(inputs)
    r1 = run_bass_kernel_spmd(nc1, s1_maps, core_ids).results
    s2_maps = stage2_inputs(r1, inputs)
    r2 = run_bass_kernel_spmd(nc2, s2_maps, core_ids).results
    return assemble_output(r2)
